# revision 13
# baseline (speedup 1.0000x reference)
"""MFGCGRU (graph-conv GRU cell) Trainium2 kernel.

Strategy: data-parallel over batch B=32 across 8 NeuronCores (4 batches
per core). All NxN supports replicated per core. Host pre-transposes
everything so the device never transposes:

  - adjacency matrices are passed as S^T [m, n] (bf16) and used as the
    *moving* matmul operand,
  - activations live feature-major: x_cat^T [66, N] with rows 0:64 = h,
    rows 64:66 = inputs (kernel rows permuted to match),
  - the diffusion conv is computed kernel-first:
        S_m @ (X @ k_m)  ==  (S_m X) k_m
    with Y_m = X @ k_m packed [node, 128] = [k_r | k_u] for the r/u pass
    (and batch-pairs for the c pass) so the PE always runs 128 wide,
  - the attention support is built unnormalized as e^T = exp(K Q^T / 8);
    its row-normalizer 1/d is applied to the e-contribution after the
    node contraction via a second PSUM accumulator.
"""

import contextlib
import os

import numpy as np
import ml_dtypes

import concourse.bass as bass
import concourse.bacc as bacc
import concourse.tile as tile
from concourse import mybir
from concourse.bass_utils import run_bass_kernel_spmd

F32 = mybir.dt.float32
BF16 = mybir.dt.bfloat16
AF = mybir.ActivationFunctionType

B, N, DIN, U, FD, SD = 32, 2048, 2, 64, 32, 64
NCORES = 8
BL = B // NCORES          # batches per core
NTW = 512                 # n-tile width
NT = N // NTW             # 4 n-tiles
NBW = 128                 # node-block width
NB = N // NBW             # 16 node blocks
FROWS = DIN + U           # 66


def _build_program():
    nc = bacc.Bacc("TRN2", debug=False, num_devices=NCORES)

    d = {}

    def din(name, shape, dt):
        d[name] = nc.dram_tensor(name, shape, dt, kind="ExternalInput").ap()

    din("xT", [BL, FROWS, N], BF16)
    din("hT", [BL, U, N], F32)
    din("a1T", [N, N], BF16)
    din("a2T", [N, N], BF16)
    din("fsT", [FD + SD, N], F32)
    din("wq", [FD, U], F32)
    din("wk", [FD, U], F32)
    din("ws1", [FD + SD, U], F32)
    din("bs1v", [U, 1], F32)
    din("ws2", [U, 1], F32)
    din("bs2v", [1, 1], F32)
    din("kkall", [FROWS, 3 * 2 * U], BF16)
    din("kk0", [FROWS, 2 * U], BF16)
    din("kcall", [FROWS, 3 * U], BF16)
    din("kc0", [FROWS, U], BF16)
    din("bru", [2 * U, 1], F32)
    din("bc2", [2 * U, 1], F32)
    out_h = nc.dram_tensor("out", [BL, U, N], F32, kind="ExternalOutput").ap()
    uscr = nc.dram_tensor("uscr", [BL, U, N], F32).ap()

    with tile.TileContext(nc) as tc:
        _emit(tc, d, out_h, uscr)
    nc.compile()
    return nc


def _emit(tc, d, out_h, uscr):
    nc = tc.nc
    ctx = contextlib.ExitStack()
    const = ctx.enter_context(tc.tile_pool(name="const", bufs=1))
    persist = ctx.enter_context(tc.tile_pool(name="persist", bufs=1))
    adjp = ctx.enter_context(tc.tile_pool(name="adjp", bufs=4))
    etp = ctx.enter_context(tc.tile_pool(name="etp", bufs=1))
    ypool = ctx.enter_context(tc.tile_pool(name="ypool", bufs=1))
    stage = ctx.enter_context(tc.tile_pool(name="stage", bufs=2))
    p3p = ctx.enter_context(tc.tile_pool(name="p3p", bufs=2))
    psacc = ctx.enter_context(tc.tile_pool(name="psacc", bufs=5, space="PSUM"))
    psscr = ctx.enter_context(tc.tile_pool(name="psscr", bufs=3, space="PSUM"))

    # ---- constants / weights in SBUF ----
    def cload(name, shape=None, dt=None):
        ap = d[name]
        t = const.tile(list(ap.shape) if shape is None else shape,
                       ap.dtype if dt is None else dt, name=f"c_{name}")
        nc.sync.dma_start(out=t, in_=ap)
        return t

    fsT = cload("fsT")
    wq = cload("wq")
    wk = cload("wk")
    ws1 = cload("ws1")
    bs1v = cload("bs1v")
    ws2 = cload("ws2")
    bs2v = cload("bs2v")

    kkall = cload("kkall")
    kcall = cload("kcall")
    kk0 = cload("kk0")
    kc0 = cload("kc0")
    bru = cload("bru")
    bc2 = cload("bc2")

    # ---- persistent activations ----
    xT = [persist.tile([FROWS, N], BF16, name=f"xT{b}", tag=f"xT{b}")
          for b in range(BL)]
    for b in range(BL):
        nc.sync.dma_start(out=xT[b], in_=d["xT"][b])

    ones_col = const.tile([128, 1], BF16, name="ones_col")
    nc.vector.memset(ones_col, 1.0)
    ones_row = const.tile([1, 128], F32, name="ones_row")
    nc.vector.memset(ones_row, 1.0)

    QT = persist.tile([U, N], BF16, name="QT", tag="QT")
    KT = persist.tile([U, N], BF16, name="KT", tag="KT")
    s_row = persist.tile([1, N], F32, name="s_row", tag="s_row")
    rdbc = [persist.tile([128, NTW], BF16, name=f"rdbc{t}", tag=f"rdbc{t}")
            for t in range(NT)]

    # ---- prelude: Q^T, K^T, s ----
    for t in range(NT):
        sl = slice(t * NTW, (t + 1) * NTW)
        pq = psscr.tile([U, NTW], F32, name="pq", tag="scr")
        nc.tensor.matmul(pq, wq, fsT[0:FD, sl], start=True, stop=True)
        nc.scalar.activation(QT[:, sl], pq, AF.Relu)
        pk = psscr.tile([U, NTW], F32, name="pk", tag="scr")
        nc.tensor.matmul(pk, wk, fsT[0:FD, sl], start=True, stop=True)
        nc.scalar.activation(KT[:, sl], pk, AF.Relu)
        ps1 = psscr.tile([U, NTW], F32, name="ps1", tag="scr")
        nc.tensor.matmul(ps1, ws1, fsT[:, sl], start=True, stop=True)
        s1t = stage.tile([U, NTW], F32, name="s1t", tag="s1t")
        nc.scalar.activation(s1t, ps1, AF.Relu, bias=bs1v)
        ps2 = psscr.tile([1, NTW], F32, name="ps2", tag="scr")
        nc.tensor.matmul(ps2, ws2, s1t, start=True, stop=True)
        nc.scalar.activation(s_row[:, sl], ps2, AF.Relu, bias=bs2v)

    # ---- phase-1 Y tiles: Y[m,b] = X_b @ [k_r[m]|k_u[m]], all m in one
    # MM: stored as [128, NB, 3, 128]: [node%128, node//128, m, u']
    y = [ypool.tile([NBW, NB, 3, 2 * U], BF16, name=f"y_{b}", tag=f"y{b}")
         for b in range(BL)]
    for b in range(BL):
        for j in range(NB):
            nsl = slice(j * NBW, (j + 1) * NBW)
            py = psscr.tile([NBW, 3 * 2 * U], F32, name="py", tag="scr")
            nc.tensor.matmul(py, xT[b][:, nsl], kkall, start=True, stop=True)
            nc.vector.tensor_copy(
                y[b][:, j, :, :],
                py.rearrange("p (m u) -> p m u", m=3))

    def e_thunks(t, et):
        """Thunks generating e^T[:, t] = exp(K Q^T / 8) into et, one
        node-block per call — interleaved into adjacency groups so the
        ACT exp evacuations hide under PE matmul streaming."""
        sl = slice(t * NTW, (t + 1) * NTW)

        def mk(j):
            def f():
                pe = psscr.tile([NBW, NTW], F32, name="pe", tag="scr")
                nc.tensor.matmul(pe, KT[:, j * NBW:(j + 1) * NBW], QT[:, sl],
                                 start=True, stop=True)
                nc.scalar.activation(et[:, j, :], pe, AF.Exp, scale=0.125)
            return f
        return [mk(j) for j in range(NB)]

    def d_thunks(et, pd):
        def mk(j):
            def f():
                nc.tensor.matmul(pd, ones_col, et[:, j, :],
                                 start=(j == 0), stop=(j == NB - 1))
            return f
        return [mk(j) for j in range(NB)]

    def interleave(main, extra, ratio=2):
        """Emit `ratio` thunks from main per one from extra."""
        mi = ei = 0
        while mi < len(main) or ei < len(extra):
            for _ in range(ratio):
                if mi < len(main):
                    main[mi](); mi += 1
            if ei < len(extra):
                extra[ei](); ei += 1

    def adjslice(name, t):
        sl = d[name][:, t * NTW:(t + 1) * NTW]
        a = adjp.tile([NBW, NB, NTW], BF16, name=f"sl_{name}", tag="adj")
        nc.sync.dma_start(out=a, in_=sl.rearrange("(j p) w -> p j w", p=NBW))
        return a

    # =================== phase 1: r & u gates ===================
    def a_thunks1(b, sl, a1, a2, pa):
        th = [lambda: nc.tensor.matmul(pa, kk0, xT[b][:, sl],
                                       start=True, stop=False)]
        for m, asl in ((0, a1), (1, a2)):
            for j in range(NB):
                def f(m=m, asl=asl, j=j):
                    nc.tensor.matmul(pa, y[b][:, j, m, :], asl[:, j, :],
                                     start=False,
                                     stop=(m == 1 and j == NB - 1))
                th.append(f)
        return th

    def agroup1(b, sl, a1, a2, extra=()):
        pa = psacc.tile([128, NTW], F32, name="pa", tag="acc")
        interleave(a_thunks1(b, sl, a1, a2, pa), list(extra))
        return pa

    def bgroup1(b, t, sl, et, pa):
        pb = psacc.tile([128, NTW], F32, name="pb", tag="acc")
        for j in range(NB):
            nc.tensor.matmul(pb, y[b][:, j, 2, :], et[:, j, :],
                             start=(j == 0), stop=(j == NB - 1))
        tmp = stage.tile([128, NTW], F32, name="tmp", tag="tmp")
        nc.vector.tensor_mul(tmp, pb, rdbc[t])
        ssum = stage.tile([128, NTW], F32, name="ssum", tag="ssum")
        nc.vector.tensor_add(ssum, pa, tmp)
        sig = stage.tile([128, NTW], F32, name="sig", tag="sig")
        nc.scalar.activation(sig, ssum, AF.Sigmoid, scale=0.25, bias=bru)
        # rh -> x_cat_c rows 0:64 in place; u -> DRAM scratch
        nc.vector.tensor_mul(xT[b][0:U, sl], sig[0:U, :], xT[b][0:U, sl])
        nc.sync.dma_start(out=uscr[b][:, sl], in_=sig[U:128, :])

    preload = (adjslice("a1T", 0), adjslice("a2T", 0))

    for t in range(NT):
        sl = slice(t * NTW, (t + 1) * NTW)
        if t == 0:
            a1, a2 = preload
        else:
            a1 = adjslice("a1T", t)
            a2 = adjslice("a2T", t)
        et = etp.tile([NBW, NB, NTW], BF16, name="et", tag="et")
        pd = psscr.tile([1, NTW], F32, name="pd", tag="scr")
        pa0 = agroup1(0, sl, a1, a2)
        pa1 = agroup1(1, sl, a1, a2, e_thunks(t, et))
        pa2 = agroup1(2, sl, a1, a2, d_thunks(et, pd))

        # d[n] = s[n] + colsum(e^T)[n]; rdbc[t][p, n] = 1 / d[n]
        dsb = stage.tile([1, NTW], F32, name="dsb", tag="dsb")
        nc.vector.tensor_add(dsb, pd, s_row[:, sl])
        rds = stage.tile([1, NTW], F32, name="rds", tag="dsb")
        nc.vector.reciprocal(rds, dsb)
        pr = psscr.tile([128, NTW], F32, name="pr", tag="scr")
        nc.tensor.matmul(pr, ones_row, rds, start=True, stop=True)
        nc.scalar.activation(rdbc[t], pr, AF.Copy)

        bgroup1(0, t, sl, et, pa0)
        bgroup1(1, t, sl, et, pa1)
        pa3 = agroup1(3, sl, a1, a2)
        bgroup1(2, t, sl, et, pa2)
        bgroup1(3, t, sl, et, pa3)

    # =================== phase 2+3: c gate & h_new ===================
    # Yc[pair] = [Xc_b0 @ kc[m] | Xc_b1 @ kc[m]] packed [128, NB, 3, 128]
    yc = [ypool.tile([NBW, NB, 3, 2 * U], BF16, name=f"yc_{p}", tag=f"y{p}")
          for p in range(BL // 2)]
    for p in range(BL // 2):
        for half in range(2):
            b = 2 * p + half
            usl = slice(half * U, (half + 1) * U)
            for j in range(NB):
                nsl = slice(j * NBW, (j + 1) * NBW)
                pyc = psscr.tile([NBW, 3 * U], F32, name="pyc", tag="scr")
                nc.tensor.matmul(pyc, xT[b][:, nsl], kcall,
                                 start=True, stop=True)
                nc.vector.tensor_copy(
                    yc[p][:, j, :, usl],
                    pyc.rearrange("p (m u) -> p m u", m=3))

    for t in range(NT):
        sl = slice(t * NTW, (t + 1) * NTW)
        a1 = adjslice("a1T", t)
        a2 = adjslice("a2T", t)
        et = etp.tile([NBW, NB, NTW], BF16, name="et2", tag="et")
        pas = []
        for p in range(BL // 2):
            b0, b1 = 2 * p, 2 * p + 1
            pa = psacc.tile([128, NTW], F32, name="pa2", tag="acc")
            th = [lambda pa=pa, p=p: nc.tensor.matmul(
                      pa, yc[p][:, 0, 0, :], a1[:, 0, :], start=True, stop=False),
                  lambda pa=pa, b0=b0: nc.tensor.matmul(
                      pa[0:U, :], kc0, xT[b0][:, sl], start=False, stop=False),
                  lambda pa=pa, b1=b1: nc.tensor.matmul(
                      pa[U:128, :], kc0, xT[b1][:, sl], start=False, stop=False)]
            for m, asl in ((0, a1), (1, a2)):
                for j in range(NB):
                    if m == 0 and j == 0:
                        continue
                    def f(pa=pa, p=p, m=m, asl=asl, j=j):
                        nc.tensor.matmul(pa, yc[p][:, j, m, :], asl[:, j, :],
                                         start=False,
                                         stop=(m == 1 and j == NB - 1))
                    th.append(f)
            interleave(th, e_thunks(t, et) if p == 0 else [])
            pas.append(pa)
        for p in range(BL // 2):
            b0, b1 = 2 * p, 2 * p + 1
            pa = pas[p]
            pb = psacc.tile([128, NTW], F32, name="pb2", tag="acc")
            for j in range(NB):
                nc.tensor.matmul(pb, yc[p][:, j, 2, :], et[:, j, :],
                                 start=(j == 0), stop=(j == NB - 1))
            tmp = stage.tile([128, NTW], F32, name="tmp2", tag="tmp")
            nc.vector.tensor_mul(tmp, pb, rdbc[t])
            ssum = stage.tile([128, NTW], F32, name="ssum2", tag="ssum")
            nc.vector.tensor_add(ssum, pa, tmp)
            ct = stage.tile([128, NTW], F32, name="ct", tag="sig")
            nc.scalar.activation(ct, ssum, AF.Tanh, scale=0.25, bias=bc2)

            # h_new = u*h + (1-u)*c = u*(h-c) + c, pair-packed [128, NTW]
            hp = p3p.tile([128, NTW], F32, name="hp", tag="hp")
            up = p3p.tile([128, NTW], F32, name="up", tag="up")
            for half, b in ((0, b0), (1, b1)):
                psl = slice(half * U, (half + 1) * U)
                nc.sync.dma_start(out=hp[psl, :], in_=d["hT"][b][:, sl])
                nc.sync.dma_start(out=up[psl, :], in_=uscr[b][:, sl])
            t1 = p3p.tile([128, NTW], F32, name="t1", tag="t1")
            nc.vector.tensor_sub(t1, hp, ct)
            nc.vector.tensor_mul(t1, up, t1)
            nc.vector.tensor_add(t1, t1, ct)
            for half, b in ((0, b0), (1, b1)):
                psl = slice(half * U, (half + 1) * U)
                nc.sync.dma_start(out=out_h[b][:, sl], in_=t1[psl, :])

    ctx.close()


_CACHE = {}


def _get_program():
    if "nc" not in _CACHE:
        _CACHE["nc"] = _build_program()
    return _CACHE["nc"]


def _prep_inputs(inputs, h_prev, adj1, adj2, feat, SE, Wq, Wk, Ws1, bs1, Ws2,
                 bs2, r_kernel, r_bias, u_kernel, u_bias, c_kernel, c_bias):
    bf = ml_dtypes.bfloat16
    f32 = np.float32
    perm = list(range(DIN, FROWS)) + list(range(DIN))  # [h(64); inputs(2)]

    h3 = np.asarray(h_prev, f32).reshape(B, N, U)
    hT = np.ascontiguousarray(h3.transpose(0, 2, 1))            # [B, U, N]
    inT = np.asarray(inputs, f32).transpose(0, 2, 1)            # [B, DIN, N]
    xT = np.concatenate([hT, inT], axis=1).astype(bf)           # [B, 66, N]

    rk = np.asarray(r_kernel, f32)[:, perm, :]
    uk = np.asarray(u_kernel, f32)[:, perm, :]
    ck = np.asarray(c_kernel, f32)[:, perm, :]
    kkall = np.concatenate(
        [np.concatenate([rk[m], uk[m]], axis=1) for m in (1, 2, 3)],
        axis=1).astype(bf)                                      # [66, 384]
    kk0 = np.concatenate([rk[0], uk[0]], axis=1).astype(bf)     # [66, 128]
    kcall = np.concatenate([ck[1], ck[2], ck[3]], axis=1).astype(bf)
    kc0 = ck[0].astype(bf)

    shared = {
        "a1T": np.ascontiguousarray(np.asarray(adj1, f32).T).astype(bf),
        "a2T": np.ascontiguousarray(np.asarray(adj2, f32).T).astype(bf),
        "fsT": np.ascontiguousarray(
            np.concatenate([np.asarray(feat, f32).T, np.asarray(SE, f32).T],
                           axis=0)),
        "wq": np.asarray(Wq, f32),
        "wk": np.asarray(Wk, f32),
        "ws1": np.asarray(Ws1, f32),
        "bs1v": np.asarray(bs1, f32).reshape(U, 1),
        "ws2": np.asarray(Ws2, f32).reshape(U, 1),
        "bs2v": np.asarray(bs2, f32).reshape(1, 1),
        "kkall": kkall,
        "kk0": kk0,
        "kcall": kcall,
        "kc0": kc0,
        "bru": np.concatenate([np.asarray(r_bias, f32).mean(0),
                               np.asarray(u_bias, f32).mean(0)]).reshape(-1, 1),
        "bc2": np.tile(np.asarray(c_bias, f32).mean(0), 2).reshape(-1, 1),
    }
    in_maps = []
    for c in range(NCORES):
        bsl = slice(c * BL, (c + 1) * BL)
        m = dict(shared)
        m["xT"] = np.ascontiguousarray(xT[bsl])
        m["hT"] = np.ascontiguousarray(hT[bsl])
        in_maps.append(m)
    return in_maps


def kernel(**inputs):
    os.environ.setdefault("NEURON_RT_RESET_CORES", "1")
    nc = _get_program()
    in_maps = _prep_inputs(**inputs)
    res = None
    err = None
    for _ in range(2):
        try:
            res = run_bass_kernel_spmd(nc, in_maps, list(range(NCORES)))
            break
        except Exception as e:  # e.g. a wedged device; retry once
            err = e
    if res is None:
        raise err
    outs = []
    for c in range(NCORES):
        o = res.results[c]["out"]                     # [BL, U, N] f32
        outs.append(o.transpose(0, 2, 1).reshape(BL, N * U))
    return np.concatenate(outs, axis=0).astype(np.float32)


# revision 14
# speedup vs baseline: 1.0165x; 1.0165x over previous
"""MFGCGRU (graph-conv GRU cell) Trainium2 kernel.

Strategy: data-parallel over batch B=32 across 8 NeuronCores (4 batches
per core). All NxN supports replicated per core. Host pre-transposes
everything so the device never transposes:

  - adjacency matrices are passed as S^T [m, n] (bf16) and used as the
    *moving* matmul operand,
  - activations live feature-major: x_cat^T [66, N] with rows 0:64 = h,
    rows 64:66 = inputs (kernel rows permuted to match),
  - the diffusion conv is computed kernel-first:
        S_m @ (X @ k_m)  ==  (S_m X) k_m
    with Y_m = X @ k_m packed [node, 128] = [k_r | k_u] for the r/u pass
    (and batch-pairs for the c pass) so the PE always runs 128 wide,
  - the attention support is built unnormalized as e^T = exp(K Q^T / 8);
    its row-normalizer 1/d is applied to the e-contribution after the
    node contraction via a second PSUM accumulator.
"""

import contextlib
import os

import numpy as np
import ml_dtypes

import concourse.bass as bass
import concourse.bacc as bacc
import concourse.tile as tile
from concourse import mybir
from concourse.bass_utils import run_bass_kernel_spmd

F32 = mybir.dt.float32
BF16 = mybir.dt.bfloat16
AF = mybir.ActivationFunctionType

B, N, DIN, U, FD, SD = 32, 2048, 2, 64, 32, 64
NCORES = 8
BL = B // NCORES          # batches per core
NTW = 512                 # n-tile width
NT = N // NTW             # 4 n-tiles
NBW = 128                 # node-block width
NB = N // NBW             # 16 node blocks
FROWS = DIN + U           # 66


def _build_program():
    nc = bacc.Bacc("TRN2", debug=False, num_devices=NCORES)

    d = {}

    def din(name, shape, dt):
        d[name] = nc.dram_tensor(name, shape, dt, kind="ExternalInput").ap()

    din("xT", [BL, FROWS, N], BF16)
    din("hT", [BL, U, N], F32)
    din("a1T", [N, N], BF16)
    din("a2T", [N, N], BF16)
    din("fsT", [FD + SD, N], F32)
    din("wq", [FD, U], F32)
    din("wk", [FD, U], F32)
    din("ws1", [FD + SD, U], F32)
    din("bs1v", [U, 1], F32)
    din("ws2", [U, 1], F32)
    din("bs2v", [1, 1], F32)
    din("kkall", [FROWS, 3 * 2 * U], BF16)
    din("kk0", [FROWS, 2 * U], BF16)
    din("kcall", [FROWS, 3 * U], BF16)
    din("kc0", [FROWS, U], BF16)
    din("bru", [2 * U, 1], F32)
    din("bc2", [2 * U, 1], F32)
    out_h = nc.dram_tensor("out", [BL, U, N], F32, kind="ExternalOutput").ap()
    uscr = nc.dram_tensor("uscr", [BL, U, N], F32).ap()

    with tile.TileContext(nc) as tc:
        _emit(tc, d, out_h, uscr)
    nc.compile()
    return nc


def _emit(tc, d, out_h, uscr):
    nc = tc.nc
    ctx = contextlib.ExitStack()
    const = ctx.enter_context(tc.tile_pool(name="const", bufs=1))
    persist = ctx.enter_context(tc.tile_pool(name="persist", bufs=1))
    adjp = ctx.enter_context(tc.tile_pool(name="adjp", bufs=4))
    etp = ctx.enter_context(tc.tile_pool(name="etp", bufs=1))
    ypool = ctx.enter_context(tc.tile_pool(name="ypool", bufs=1))
    stage = ctx.enter_context(tc.tile_pool(name="stage", bufs=2))
    p3p = ctx.enter_context(tc.tile_pool(name="p3p", bufs=2))
    psacc = ctx.enter_context(tc.tile_pool(name="psacc", bufs=5, space="PSUM"))
    psscr = ctx.enter_context(tc.tile_pool(name="psscr", bufs=3, space="PSUM"))

    # ---- constants / weights in SBUF ----
    def cload(name, shape=None, dt=None):
        ap = d[name]
        t = const.tile(list(ap.shape) if shape is None else shape,
                       ap.dtype if dt is None else dt, name=f"c_{name}")
        nc.sync.dma_start(out=t, in_=ap)
        return t

    fsT = const.tile([FD + SD, N], F32, name="c_fsT")
    nc.sync.dma_start(out=fsT[0:FD, :], in_=d["fsT"][0:FD, :])
    wq = cload("wq")
    wk = cload("wk")
    nc.sync.dma_start(out=fsT[FD:, :], in_=d["fsT"][FD:, :])
    ws1 = cload("ws1")
    bs1v = cload("bs1v")
    ws2 = cload("ws2")
    bs2v = cload("bs2v")

    kkall = cload("kkall")
    kcall = cload("kcall")
    kk0 = cload("kk0")
    kc0 = cload("kc0")
    bru = cload("bru")
    bc2 = cload("bc2")

    # ---- persistent activations ----
    xT = [persist.tile([FROWS, N], BF16, name=f"xT{b}", tag=f"xT{b}")
          for b in range(BL)]
    for b in range(BL):
        nc.sync.dma_start(out=xT[b], in_=d["xT"][b])

    ones_col = const.tile([128, 1], BF16, name="ones_col")
    nc.vector.memset(ones_col, 1.0)
    ones_row = const.tile([1, 128], F32, name="ones_row")
    nc.vector.memset(ones_row, 1.0)

    QT = persist.tile([U, N], BF16, name="QT", tag="QT")
    KT = persist.tile([U, N], BF16, name="KT", tag="KT")
    s_row = persist.tile([1, N], F32, name="s_row", tag="s_row")
    rdbc = [persist.tile([128, NTW], BF16, name=f"rdbc{t}", tag=f"rdbc{t}")
            for t in range(NT)]

    # ---- prelude: Q^T, K^T, s ----
    for t in range(NT):
        sl = slice(t * NTW, (t + 1) * NTW)
        pq = psscr.tile([U, NTW], F32, name="pq", tag="scr")
        nc.tensor.matmul(pq, wq, fsT[0:FD, sl], start=True, stop=True)
        nc.scalar.activation(QT[:, sl], pq, AF.Relu)
        pk = psscr.tile([U, NTW], F32, name="pk", tag="scr")
        nc.tensor.matmul(pk, wk, fsT[0:FD, sl], start=True, stop=True)
        nc.scalar.activation(KT[:, sl], pk, AF.Relu)
        ps1 = psscr.tile([U, NTW], F32, name="ps1", tag="scr")
        nc.tensor.matmul(ps1, ws1, fsT[:, sl], start=True, stop=True)
        s1t = stage.tile([U, NTW], F32, name="s1t", tag="s1t")
        nc.scalar.activation(s1t, ps1, AF.Relu, bias=bs1v)
        ps2 = psscr.tile([1, NTW], F32, name="ps2", tag="scr")
        nc.tensor.matmul(ps2, ws2, s1t, start=True, stop=True)
        nc.scalar.activation(s_row[:, sl], ps2, AF.Relu, bias=bs2v)

    # ---- phase-1 Y tiles: Y[m,b] = X_b @ [k_r[m]|k_u[m]], all m in one
    # MM: stored as [128, NB, 3, 128]: [node%128, node//128, m, u']
    y = [ypool.tile([NBW, NB, 3, 2 * U], BF16, name=f"y_{b}", tag=f"y{b}")
         for b in range(BL)]
    for b in range(BL):
        for j in range(NB):
            nsl = slice(j * NBW, (j + 1) * NBW)
            py = psscr.tile([NBW, 3 * 2 * U], F32, name="py", tag="scr")
            nc.tensor.matmul(py, xT[b][:, nsl], kkall, start=True, stop=True)
            nc.vector.tensor_copy(
                y[b][:, j, :, :],
                py.rearrange("p (m u) -> p m u", m=3))

    def e_thunks(t, et):
        """Thunks generating e^T[:, t] = exp(K Q^T / 8) into et, one
        node-block per call — interleaved into adjacency groups so the
        ACT exp evacuations hide under PE matmul streaming."""
        sl = slice(t * NTW, (t + 1) * NTW)

        def mk(j):
            def f():
                pe = psscr.tile([NBW, NTW], F32, name="pe", tag="scr")
                nc.tensor.matmul(pe, KT[:, j * NBW:(j + 1) * NBW], QT[:, sl],
                                 start=True, stop=True)
                nc.scalar.activation(et[:, j, :], pe, AF.Exp, scale=0.125)
            return f
        return [mk(j) for j in range(NB)]

    def d_thunks(et, pd):
        def mk(j):
            def f():
                nc.tensor.matmul(pd, ones_col, et[:, j, :],
                                 start=(j == 0), stop=(j == NB - 1))
            return f
        return [mk(j) for j in range(NB)]

    def interleave(main, extra, ratio=2):
        """Emit `ratio` thunks from main per one from extra."""
        mi = ei = 0
        while mi < len(main) or ei < len(extra):
            for _ in range(ratio):
                if mi < len(main):
                    main[mi](); mi += 1
            if ei < len(extra):
                extra[ei](); ei += 1

    def adjslice(name, t):
        sl = d[name][:, t * NTW:(t + 1) * NTW]
        a = adjp.tile([NBW, NB, NTW], BF16, name=f"sl_{name}", tag="adj")
        nc.sync.dma_start(out=a, in_=sl.rearrange("(j p) w -> p j w", p=NBW))
        return a

    # =================== phase 1: r & u gates ===================
    def a_thunks1(b, sl, a1, a2, pa):
        th = [lambda: nc.tensor.matmul(pa, kk0, xT[b][:, sl],
                                       start=True, stop=False)]
        for m, asl in ((0, a1), (1, a2)):
            for j in range(NB):
                def f(m=m, asl=asl, j=j):
                    nc.tensor.matmul(pa, y[b][:, j, m, :], asl[:, j, :],
                                     start=False,
                                     stop=(m == 1 and j == NB - 1))
                th.append(f)
        return th

    def agroup1(b, sl, a1, a2, extra=()):
        pa = psacc.tile([128, NTW], F32, name="pa", tag="acc")
        interleave(a_thunks1(b, sl, a1, a2, pa), list(extra))
        return pa

    def bgroup1(b, t, sl, et, pa):
        pb = psacc.tile([128, NTW], F32, name="pb", tag="acc")
        for j in range(NB):
            nc.tensor.matmul(pb, y[b][:, j, 2, :], et[:, j, :],
                             start=(j == 0), stop=(j == NB - 1))
        tmp = stage.tile([128, NTW], F32, name="tmp", tag="tmp")
        nc.vector.tensor_mul(tmp, pb, rdbc[t])
        ssum = stage.tile([128, NTW], F32, name="ssum", tag="ssum")
        nc.vector.tensor_add(ssum, pa, tmp)
        sig = stage.tile([128, NTW], F32, name="sig", tag="sig")
        nc.scalar.activation(sig, ssum, AF.Sigmoid, scale=0.25, bias=bru)
        # rh -> x_cat_c rows 0:64 in place; u -> DRAM scratch
        nc.vector.tensor_mul(xT[b][0:U, sl], sig[0:U, :], xT[b][0:U, sl])
        nc.sync.dma_start(out=uscr[b][:, sl], in_=sig[U:128, :])

    preload = (adjslice("a1T", 0), adjslice("a2T", 0))

    for t in range(NT):
        sl = slice(t * NTW, (t + 1) * NTW)
        if t == 0:
            a1, a2 = preload
        else:
            a1 = adjslice("a1T", t)
            a2 = adjslice("a2T", t)
        et = etp.tile([NBW, NB, NTW], BF16, name="et", tag="et")
        pd = psscr.tile([1, NTW], F32, name="pd", tag="scr")
        pa0 = agroup1(0, sl, a1, a2)
        pa1 = agroup1(1, sl, a1, a2, e_thunks(t, et))
        pa2 = agroup1(2, sl, a1, a2, d_thunks(et, pd))

        # d[n] = s[n] + colsum(e^T)[n]; rdbc[t][p, n] = 1 / d[n]
        dsb = stage.tile([1, NTW], F32, name="dsb", tag="dsb")
        nc.vector.tensor_add(dsb, pd, s_row[:, sl])
        rds = stage.tile([1, NTW], F32, name="rds", tag="dsb")
        nc.vector.reciprocal(rds, dsb)
        pr = psscr.tile([128, NTW], F32, name="pr", tag="scr")
        nc.tensor.matmul(pr, ones_row, rds, start=True, stop=True)
        nc.scalar.activation(rdbc[t], pr, AF.Copy)

        bgroup1(0, t, sl, et, pa0)
        bgroup1(1, t, sl, et, pa1)
        pa3 = agroup1(3, sl, a1, a2)
        bgroup1(2, t, sl, et, pa2)
        bgroup1(3, t, sl, et, pa3)

    # =================== phase 2+3: c gate & h_new ===================
    # Yc[pair] = [Xc_b0 @ kc[m] | Xc_b1 @ kc[m]] packed [128, NB, 3, 128]
    yc = [ypool.tile([NBW, NB, 3, 2 * U], BF16, name=f"yc_{p}", tag=f"y{p}")
          for p in range(BL // 2)]
    for p in range(BL // 2):
        for half in range(2):
            b = 2 * p + half
            usl = slice(half * U, (half + 1) * U)
            for j in range(NB):
                nsl = slice(j * NBW, (j + 1) * NBW)
                pyc = psscr.tile([NBW, 3 * U], F32, name="pyc", tag="scr")
                nc.tensor.matmul(pyc, xT[b][:, nsl], kcall,
                                 start=True, stop=True)
                nc.vector.tensor_copy(
                    yc[p][:, j, :, usl],
                    pyc.rearrange("p (m u) -> p m u", m=3))

    for t in range(NT):
        sl = slice(t * NTW, (t + 1) * NTW)
        a1 = adjslice("a1T", t)
        a2 = adjslice("a2T", t)
        et = etp.tile([NBW, NB, NTW], BF16, name="et2", tag="et")
        pas = []
        for p in range(BL // 2):
            b0, b1 = 2 * p, 2 * p + 1
            pa = psacc.tile([128, NTW], F32, name="pa2", tag="acc")
            th = [lambda pa=pa, p=p: nc.tensor.matmul(
                      pa, yc[p][:, 0, 0, :], a1[:, 0, :], start=True, stop=False),
                  lambda pa=pa, b0=b0: nc.tensor.matmul(
                      pa[0:U, :], kc0, xT[b0][:, sl], start=False, stop=False),
                  lambda pa=pa, b1=b1: nc.tensor.matmul(
                      pa[U:128, :], kc0, xT[b1][:, sl], start=False, stop=False)]
            for m, asl in ((0, a1), (1, a2)):
                for j in range(NB):
                    if m == 0 and j == 0:
                        continue
                    def f(pa=pa, p=p, m=m, asl=asl, j=j):
                        nc.tensor.matmul(pa, yc[p][:, j, m, :], asl[:, j, :],
                                         start=False,
                                         stop=(m == 1 and j == NB - 1))
                    th.append(f)
            interleave(th, e_thunks(t, et) if p == 0 else [])
            pas.append(pa)
        for p in range(BL // 2):
            b0, b1 = 2 * p, 2 * p + 1
            pa = pas[p]
            # prefetch h and u for the tail chain
            hp = p3p.tile([128, NTW], F32, name="hp", tag="hp")
            up = p3p.tile([128, NTW], F32, name="up", tag="up")
            for half, b in ((0, b0), (1, b1)):
                psl = slice(half * U, (half + 1) * U)
                nc.sync.dma_start(out=hp[psl, :], in_=d["hT"][b][:, sl])
                nc.sync.dma_start(out=up[psl, :], in_=uscr[b][:, sl])
            pb = psacc.tile([128, NTW], F32, name="pb2", tag="acc")
            for j in range(NB):
                nc.tensor.matmul(pb, yc[p][:, j, 2, :], et[:, j, :],
                                 start=(j == 0), stop=(j == NB - 1))
            tmp = stage.tile([128, NTW], F32, name="tmp2", tag="tmp")
            ct = stage.tile([128, NTW], F32, name="ct", tag="sig")
            t1 = p3p.tile([128, NTW], F32, name="t1", tag="t1")
            # run the gate + elementwise chain in column halves so the
            # DVE / ACT / DMA stages pipeline instead of serializing
            for c0 in range(0, NTW, NTW // 2):
                cs = slice(c0, c0 + NTW // 2)
                nc.vector.tensor_mul(tmp[:, cs], pb[:, cs], rdbc[t][:, cs])
                nc.vector.tensor_add(tmp[:, cs], pa[:, cs], tmp[:, cs])
                nc.scalar.activation(ct[:, cs], tmp[:, cs], AF.Tanh,
                                     scale=0.25, bias=bc2)
                nc.vector.tensor_sub(t1[:, cs], hp[:, cs], ct[:, cs])
                nc.vector.tensor_mul(t1[:, cs], up[:, cs], t1[:, cs])
                nc.vector.tensor_add(t1[:, cs], t1[:, cs], ct[:, cs])
                for half, b in ((0, b0), (1, b1)):
                    psl = slice(half * U, (half + 1) * U)
                    nc.sync.dma_start(
                        out=out_h[b][:, t * NTW + c0:t * NTW + c0 + NTW // 2],
                        in_=t1[psl, cs])

    ctx.close()


_CACHE = {}


def _get_program():
    if "nc" not in _CACHE:
        _CACHE["nc"] = _build_program()
    return _CACHE["nc"]


def _prep_inputs(inputs, h_prev, adj1, adj2, feat, SE, Wq, Wk, Ws1, bs1, Ws2,
                 bs2, r_kernel, r_bias, u_kernel, u_bias, c_kernel, c_bias):
    bf = ml_dtypes.bfloat16
    f32 = np.float32
    perm = list(range(DIN, FROWS)) + list(range(DIN))  # [h(64); inputs(2)]

    h3 = np.asarray(h_prev, f32).reshape(B, N, U)
    hT = np.ascontiguousarray(h3.transpose(0, 2, 1))            # [B, U, N]
    inT = np.asarray(inputs, f32).transpose(0, 2, 1)            # [B, DIN, N]
    xT = np.concatenate([hT, inT], axis=1).astype(bf)           # [B, 66, N]

    rk = np.asarray(r_kernel, f32)[:, perm, :]
    uk = np.asarray(u_kernel, f32)[:, perm, :]
    ck = np.asarray(c_kernel, f32)[:, perm, :]
    kkall = np.concatenate(
        [np.concatenate([rk[m], uk[m]], axis=1) for m in (1, 2, 3)],
        axis=1).astype(bf)                                      # [66, 384]
    kk0 = np.concatenate([rk[0], uk[0]], axis=1).astype(bf)     # [66, 128]
    kcall = np.concatenate([ck[1], ck[2], ck[3]], axis=1).astype(bf)
    kc0 = ck[0].astype(bf)

    shared = {
        "a1T": np.ascontiguousarray(np.asarray(adj1, f32).T).astype(bf),
        "a2T": np.ascontiguousarray(np.asarray(adj2, f32).T).astype(bf),
        "fsT": np.ascontiguousarray(
            np.concatenate([np.asarray(feat, f32).T, np.asarray(SE, f32).T],
                           axis=0)),
        "wq": np.asarray(Wq, f32),
        "wk": np.asarray(Wk, f32),
        "ws1": np.asarray(Ws1, f32),
        "bs1v": np.asarray(bs1, f32).reshape(U, 1),
        "ws2": np.asarray(Ws2, f32).reshape(U, 1),
        "bs2v": np.asarray(bs2, f32).reshape(1, 1),
        "kkall": kkall,
        "kk0": kk0,
        "kcall": kcall,
        "kc0": kc0,
        "bru": np.concatenate([np.asarray(r_bias, f32).mean(0),
                               np.asarray(u_bias, f32).mean(0)]).reshape(-1, 1),
        "bc2": np.tile(np.asarray(c_bias, f32).mean(0), 2).reshape(-1, 1),
    }
    in_maps = []
    for c in range(NCORES):
        bsl = slice(c * BL, (c + 1) * BL)
        m = dict(shared)
        m["xT"] = np.ascontiguousarray(xT[bsl])
        m["hT"] = np.ascontiguousarray(hT[bsl])
        in_maps.append(m)
    return in_maps


def kernel(**inputs):
    os.environ.setdefault("NEURON_RT_RESET_CORES", "1")
    nc = _get_program()
    in_maps = _prep_inputs(**inputs)
    res = None
    err = None
    for _ in range(2):
        try:
            res = run_bass_kernel_spmd(nc, in_maps, list(range(NCORES)))
            break
        except Exception as e:  # e.g. a wedged device; retry once
            err = e
    if res is None:
        raise err
    outs = []
    for c in range(NCORES):
        o = res.results[c]["out"]                     # [BL, U, N] f32
        outs.append(o.transpose(0, 2, 1).reshape(BL, N * U))
    return np.concatenate(outs, axis=0).astype(np.float32)
